# revision 7
# baseline (speedup 1.0000x reference)
"""MLA (multi-head latent attention) Trainium2 kernel.

Sharding: 8 cores = 4-way tensor-parallel over heads x 2-way data-parallel
over batch. Each core computes, for its batch b and its 4 heads:
  qT = (x @ Wq_shard).T          per head [D=128, S]   (transposed layout)
  latT = (x @ Wkv).T             [DL=512, S]
  kT = (lat @ Wk_shard).T        per head [128, S]     (Wk = host-fused [Wkr|Wkn])
  v = lat @ Wv_shard             [S, 4*128]            (natural layout)
  q/k RMSNorm in transposed layout: sumsq via ones-column matmul, rsqrt row,
    broadcast across partitions with gpsimd.partition_broadcast, DVE multiply
  scoresT[k,q]: kT tile stationary, qT moving
  e = exp(scoresT)  (no max subtraction: |scores| <= sqrt(D) ~ 11.3, safe in fp32)
  softmax denominators via ones-column matmul (partition reduction in PSUM)
  oT[d,q] = v.T @ e (PSUM accumulation over k tiles), normalized by 1/sums
  out_partial = oT.T @ Wo_shard  [S, HID]  (natural layout, fp32),
    interleaved with attention per q-block for overlap
Host sums the 4 TP partials per batch.
"""

import sys

if "/opt/trn_rl_repo" not in sys.path:
    sys.path.insert(0, "/opt/trn_rl_repo")

import numpy as np
import ml_dtypes

BF16 = ml_dtypes.bfloat16

H = 16       # total heads
D = 128      # head dim
R = 64       # rope part of head dim
HID = 2048   # hidden
DL = 512     # kv latent
S = 2048     # seq len
B = 2        # batch
EPS = 1e-6

TP = 4                # head-parallel ways
DP = 2                # batch-parallel ways
HPC = H // TP         # heads per core = 4
NKT = S // 128        # 16 k tiles of 128
NQB = S // 512        # 4 q blocks of 512
KT_HID = HID // 128   # 16 contraction tiles for HID
KT_DL = DL // 128     # 4 contraction tiles for DL

_PROGRAMS = {}


def _build_program(repeat=1, apply_g=False):
    import concourse.bass as bass
    from concourse import bacc, mybir
    import concourse.tile as tile

    f32 = mybir.dt.float32
    bf16 = mybir.dt.bfloat16
    Exp = mybir.ActivationFunctionType.Exp
    Sqrt = mybir.ActivationFunctionType.Sqrt

    nc = bacc.Bacc(
        "TRN2", target_bir_lowering=False, debug=False, enable_asserts=False
    )

    xT_d = nc.dram_tensor("xt", [HID, S], bf16, kind="ExternalInput").ap()
    wq_d = nc.dram_tensor("wq", [HID, HPC * D], bf16, kind="ExternalInput").ap()
    wkv_d = nc.dram_tensor("wkv", [HID, DL], bf16, kind="ExternalInput").ap()
    wk_d = nc.dram_tensor("wk", [DL, HPC * D], bf16, kind="ExternalInput").ap()
    wv_d = nc.dram_tensor("wv", [DL, HPC * D], bf16, kind="ExternalInput").ap()
    wo_d = nc.dram_tensor("wo", [HPC * D, HID], bf16, kind="ExternalInput").ap()
    onec_d = nc.dram_tensor("onecol", [128, 1], bf16, kind="ExternalInput").ap()
    cmask_d = nc.dram_tensor("cmask", [128, 4, 512], bf16, kind="ExternalInput").ap()
    out_d = nc.dram_tensor("out", [S, HID], f32, kind="ExternalOutput").ap()
    if apply_g:
        gqc_d = nc.dram_tensor("gqcol", [128, 1], f32, kind="ExternalInput").ap()
        gkc_d = nc.dram_tensor("gkcol", [128, 1], f32, kind="ExternalInput").ap()

    with tile.TileContext(nc) as tc:
        with (
            tc.tile_pool(name="consts", bufs=1) as consts,
            tc.tile_pool(name="acts", bufs=1) as acts,
        ):
            # eps biases for the rsqrt rows:
            #  q: rsqrt(sumsq + D*eps) == rsqrt(mean+eps)/sqrt(D)  (attn scale folded)
            #  k: rsqrt(sumsq/D + eps) == rsqrt(mean+eps)
            epsq_sb = consts.tile([1, 1], f32)
            nc.vector.memset(epsq_sb, EPS * D)
            epsk_sb = consts.tile([1, 1], f32)
            nc.vector.memset(epsk_sb, EPS)
            onec_sb = consts.tile_from(onec_d)
            cmask_sb = consts.tile_from(cmask_d)
            if apply_g:
                gqc_sb = consts.tile_from(gqc_d)
                gkc_sb = consts.tile_from(gkc_d)

            qT = acts.tile([128, HPC, S], bf16)
            kT = acts.tile([128, HPC, S], bf16)
            latT = acts.tile([128, KT_DL, S], bf16)
            v = acts.tile([128, NKT, HPC * D], bf16)
            oT = acts.tile([128, HPC, S], bf16)

            for _rep in range(repeat):
                # ------------ stage 1: x-projections ------------
                with (
                    tc.tile_pool(name="xw", bufs=1) as xw,
                    tc.tile_pool(name="ps1", bufs=1, space="PSUM") as ps1,
                ):
                    xT_sb = xw.tile([128, KT_HID, S], bf16)
                    wq_sb = xw.tile([128, KT_HID, HPC * D], bf16)
                    wkv_sb = xw.tile([128, KT_HID, DL], bf16)
                    xr = xT_d.rearrange("(t p) s -> p t s", p=128)
                    wqr = wq_d.rearrange("(t p) n -> p t n", p=128)
                    wkvr = wkv_d.rearrange("(t p) n -> p t n", p=128)
                    for k in range(KT_HID):
                        nc.sync.dma_start(out=wq_sb[:, k, :], in_=wqr[:, k, :])
                        nc.sync.dma_start(out=wkv_sb[:, k, :], in_=wkvr[:, k, :])
                        nc.sync.dma_start(out=xT_sb[:, k, :], in_=xr[:, k, :])

                    for sb in range(NQB):
                        pq = [
                            ps1.tile([128, 512], f32, name=f"pq{n}", tag=f"pq{n}")
                            for n in range(HPC)
                        ]
                        pl = [
                            ps1.tile([128, 512], f32, name=f"pl{n}", tag=f"pl{n}")
                            for n in range(HPC)
                        ]
                        ss = slice(sb * 512, (sb + 1) * 512)
                        for k in range(KT_HID):
                            st, sp = (k == 0), (k == KT_HID - 1)
                            for n in range(HPC):
                                nc.tensor.matmul(
                                    pq[n],
                                    lhsT=wq_sb[:, k, n * 128 : (n + 1) * 128],
                                    rhs=xT_sb[:, k, ss],
                                    start=st, stop=sp,
                                )
                            for n in range(HPC):
                                nc.tensor.matmul(
                                    pl[n],
                                    lhsT=wkv_sb[:, k, n * 128 : (n + 1) * 128],
                                    rhs=xT_sb[:, k, ss],
                                    start=st, stop=sp,
                                )
                        for n in range(HPC):
                            nc.scalar.copy(out=qT[:, n, ss], in_=pq[n])
                            nc.vector.tensor_copy(out=latT[:, n, ss], in_=pl[n])

                # weights for later stages: loaded into space freed by stage 1
                with (
                    tc.tile_pool(name="w2", bufs=1) as w2,
                    tc.tile_pool(name="work", bufs=4) as work,
                    tc.tile_pool(name="nrow", bufs=4) as nrow,
                    tc.tile_pool(name="bcp", bufs=4) as bcp,
                ):
                    wk_sb = w2.tile_from(wk_d.rearrange("(t p) n -> p t n", p=128))
                    wv_sb = w2.tile_from(wv_d.rearrange("(t p) n -> p t n", p=128))
                    wo_sb = w2.tile_from(wo_d.rearrange("(t p) n -> p t n", p=128))

                    # ------------ stage 2: latent projections ------------
                    with (
                        tc.tile_pool(name="psk2", bufs=1, space="PSUM") as psk2,
                        tc.tile_pool(name="psv2", bufs=2, space="PSUM") as psv2,
                    ):
                        # kT per sb (all heads), earliest-needed first
                        for sb in range(NQB):
                            pk = [
                                psk2.tile([128, 512], f32, name=f"pk{h}", tag=f"pk{h}")
                                for h in range(HPC)
                            ]
                            ss = slice(sb * 512, (sb + 1) * 512)
                            for c in range(KT_DL):
                                for h in range(HPC):
                                    nc.tensor.matmul(
                                        pk[h],
                                        lhsT=wk_sb[:, c, h * 128 : (h + 1) * 128],
                                        rhs=latT[:, c, ss],
                                        start=(c == 0), stop=(c == KT_DL - 1),
                                    )
                            for h in range(HPC):
                                nc.scalar.copy(out=kT[:, h, ss], in_=pk[h])
                            # v for this sb's 4 s-tiles
                            for st in range(4 * sb, 4 * sb + 4):
                                pv = psv2.tile(
                                    [128, 512], f32, name=f"pv{st % 2}",
                                    tag=f"pv{st % 2}",
                                )
                                for c in range(KT_DL):
                                    nc.tensor.matmul(
                                        pv,
                                        lhsT=latT[:, c, st * 128 : (st + 1) * 128],
                                        rhs=wv_sb[:, c, :],
                                        start=(c == 0), stop=(c == KT_DL - 1),
                                    )
                                nc.vector.tensor_copy(out=v[:, st, :], in_=pv)

                    # ------------ stage 3: q/k RMSNorm ------------
                    with tc.tile_pool(name="psn", bufs=2, space="PSUM") as psn:
                        for sb in range(NQB):
                            ss = slice(sb * 512, (sb + 1) * 512)
                            for t_sb, eps_b, g_sb in (
                                (qT, epsq_sb, "gq"),
                                (kT, epsk_sb, "gk"),
                            ):
                                for h in range(HPC):
                                    sl = t_sb[:, h, ss]
                                    sq = work.tile([128, 512], bf16, tag="sq")
                                    nc.vector.tensor_mul(sq, sl, sl)
                                    pss = psn.tile([1, 512], f32)
                                    nc.tensor.matmul(
                                        pss, lhsT=onec_sb, rhs=sq,
                                        start=True, stop=True,
                                    )
                                    row = nrow.tile([1, 512], f32, tag="row")
                                    scale = 1.0 if t_sb is qT else 1.0 / D
                                    nc.scalar.activation(
                                        out=row, in_=pss, func=Sqrt,
                                        bias=eps_b, scale=scale,
                                    )
                                    nc.vector.reciprocal(out=row, in_=row)
                                    bc = bcp.tile([128, 512], f32, tag="bc")
                                    nc.gpsimd.partition_broadcast(bc, row)
                                    nc.vector.tensor_mul(sl, sl, bc)
                                    if apply_g:
                                        gc = gqc_sb if g_sb == "gq" else gkc_sb
                                        nc.vector.tensor_scalar_mul(sl, sl, gc)

                    # ------------ stages 4+5: attention + out projection ----
                    with (
                        tc.tile_pool(name="epool", bufs=6) as epool,
                        tc.tile_pool(name="otile", bufs=4) as otile,
                        tc.tile_pool(name="pss", bufs=2, space="PSUM") as pss_p,
                        tc.tile_pool(name="pso", bufs=2, space="PSUM") as pso_p,
                        tc.tile_pool(name="pssum", bufs=1, space="PSUM") as pssum_p,
                        tc.tile_pool(name="ps5", bufs=1, space="PSUM") as ps5,
                    ):
                        for j in range(NQB):  # q block of 512
                            nk = 4 * j + 4  # valid k tiles (causal)
                            js = slice(j * 512, (j + 1) * 512)
                            for h in range(HPC):
                                po = pso_p.tile([128, 512], f32, name="po")
                                psum = pssum_p.tile([1, 512], f32, name="psum")
                                qs = qT[:, h, js]
                                for t in range(nk):
                                    ps = pss_p.tile([128, 512], f32, name="ps")
                                    nc.tensor.matmul(
                                        ps,
                                        lhsT=kT[:, h, t * 128 : (t + 1) * 128],
                                        rhs=qs, start=True, stop=True,
                                    )
                                    e = epool.tile([128, 512], bf16, tag="e")
                                    nc.scalar.activation(out=e, in_=ps, func=Exp)
                                    m = t - 4 * j
                                    if m >= 0:  # diagonal band: causal mask
                                        nc.vector.tensor_mul(e, e, cmask_sb[:, m, :])
                                    nc.tensor.matmul(
                                        psum, lhsT=onec_sb, rhs=e,
                                        start=(t == 0), stop=(t == nk - 1),
                                    )
                                    nc.tensor.matmul(
                                        po,
                                        lhsT=v[:, t, h * 128 : (h + 1) * 128],
                                        rhs=e, start=(t == 0), stop=(t == nk - 1),
                                    )
                                srow = nrow.tile([1, 512], f32, tag="sr")
                                nc.vector.reciprocal(out=srow, in_=psum)
                                bc = bcp.tile([128, 512], f32, tag="abc")
                                nc.gpsimd.partition_broadcast(bc, srow)
                                nc.vector.tensor_mul(oT[:, h, js], po, bc)
                            # out projection for this q block (st = 4j..4j+3)
                            for st in range(4 * j, 4 * j + 4):
                                for nbp in range(2):  # nb pairs share 2 psum banks
                                    px = [
                                        ps5.tile(
                                            [128, 512], f32,
                                            name=f"px{i}", tag=f"px{i}",
                                        )
                                        for i in range(2)
                                    ]
                                    for c in range(HPC):
                                        for i in range(2):
                                            nb = 2 * nbp + i
                                            nc.tensor.matmul(
                                                px[i],
                                                lhsT=oT[:, c, st * 128 : (st + 1) * 128],
                                                rhs=wo_sb[:, c, nb * 512 : (nb + 1) * 512],
                                                start=(c == 0), stop=(c == HPC - 1),
                                            )
                                    for i in range(2):
                                        nb = 2 * nbp + i
                                        ot = otile.tile(
                                            [128, 512], f32, name=f"ot{i}",
                                            tag=f"ot{i}",
                                        )
                                        if i == 0:
                                            nc.vector.tensor_copy(out=ot, in_=px[i])
                                        else:
                                            nc.scalar.copy(out=ot, in_=px[i])
                                        nc.sync.dma_start(
                                            out=out_d[
                                                st * 128 : (st + 1) * 128,
                                                nb * 512 : (nb + 1) * 512,
                                            ],
                                            in_=ot,
                                        )

    nc.compile()
    return nc


def _get_program(repeat=1, apply_g=False):
    key = (repeat, apply_g)
    if key not in _PROGRAMS:
        _PROGRAMS[key] = _build_program(repeat, apply_g)
    return _PROGRAMS[key]


def _host_prep(x, Wq, Wkv, Wkr, Wkn, Wv, Wo, gq, gk):
    """Build the 8 per-core input maps. Returns (in_maps, apply_g)."""
    x = np.asarray(x, np.float32)
    gq = np.asarray(gq, np.float32)
    gk = np.asarray(gk, np.float32)
    apply_g = not (np.allclose(gq, 1.0) and np.allclose(gk, 1.0))

    # Fuse Wkr/Wkn so head h's kT block is 128 contiguous cols.
    Wk = np.empty((DL, H * D), np.float32)
    Wkr = np.asarray(Wkr, np.float32).reshape(DL, H, R)
    Wkn = np.asarray(Wkn, np.float32).reshape(DL, H, D - R)
    Wk_r = Wk.reshape(DL, H, D)
    Wk_r[:, :, :R] = Wkr
    Wk_r[:, :, R:] = Wkn

    Wq16 = np.ascontiguousarray(np.asarray(Wq, np.float32)).astype(BF16)
    Wkv16 = np.ascontiguousarray(np.asarray(Wkv, np.float32)).astype(BF16)
    Wk16 = Wk.astype(BF16)
    Wv16 = np.ascontiguousarray(np.asarray(Wv, np.float32)).astype(BF16)
    Wo16 = np.ascontiguousarray(np.asarray(Wo, np.float32)).astype(BF16)

    xT16 = [np.ascontiguousarray(x[b].T).astype(BF16) for b in range(B)]

    onec = np.ones((128, 1), BF16)
    kk = np.arange(128).reshape(128, 1, 1)
    mm = np.arange(4).reshape(1, 4, 1)
    qq = np.arange(512).reshape(1, 1, 512)
    cmask = (qq >= kk + 128 * mm).astype(BF16)  # [128, 4, 512]

    in_maps = []
    for core in range(8):
        tp, dp = core % TP, core // TP
        hs = slice(tp * HPC * D, (tp + 1) * HPC * D)
        m = {
            "xt": xT16[dp],
            "wq": np.ascontiguousarray(Wq16[:, hs]),
            "wkv": Wkv16,
            "wk": np.ascontiguousarray(Wk16[:, hs]),
            "wv": np.ascontiguousarray(Wv16[:, hs]),
            "wo": np.ascontiguousarray(Wo16[hs, :]),
            "onecol": onec,
            "cmask": cmask,
        }
        if apply_g:
            m["gqcol"] = gq.reshape(128, 1).astype(np.float32)
            m["gkcol"] = gk.reshape(128, 1).astype(np.float32)
        in_maps.append(m)
    return in_maps, apply_g


def kernel(x, Wq, Wkv, Wkr, Wkn, Wv, Wo, gq, gk, _trace=False):
    from concourse import bass_utils

    in_maps, apply_g = _host_prep(x, Wq, Wkv, Wkr, Wkn, Wv, Wo, gq, gk)
    nc = _get_program(1, apply_g)
    res = bass_utils.run_bass_kernel_spmd(
        nc, in_maps, core_ids=list(range(8)), trace=_trace
    )
    out = np.zeros((B, S, HID), np.float32)
    for core in range(8):
        out[core // TP] += np.asarray(res.results[core]["out"], np.float32)
    if _trace:
        kernel._last_result = res
    return out


# revision 10
# speedup vs baseline: 1.6180x; 1.6180x over previous
"""MLA (multi-head latent attention) Trainium2 kernel.

Sharding: 8 cores = 4-way tensor-parallel over heads x 2-way data-parallel
over batch. Each core computes, for its batch b and its 4 heads:
  qT = (x @ Wq_shard).T          per head [D=128, S]   (transposed layout)
  latT = (x @ Wkv).T             [DL=512, S]
  kT = (lat @ Wk_shard).T        per head [128, S]     (Wk = host-fused [Wkr|Wkn])
  v = lat @ Wv_shard             [S, 4*128]            (natural layout)
  q/k RMSNorm in transposed layout: sumsq via ones-column matmul, rsqrt row,
    broadcast across partitions with gpsimd.partition_broadcast, DVE multiply
  scoresT[k,q]: kT tile stationary, qT moving
  e = exp(scoresT)  (no max subtraction: |scores| <= sqrt(D) ~ 11.3, safe in fp32)
  softmax denominators via ones-column matmul (partition reduction in PSUM)
  oT[d,q] = v.T @ e (PSUM accumulation over k tiles), normalized by 1/sums
  out_partial = oT.T @ Wo_shard  [S, HID]  (natural layout, fp32),
    interleaved with attention per q-block for overlap
Host sums the 4 TP partials per batch.
"""

import sys

if "/opt/trn_rl_repo" not in sys.path:
    sys.path.insert(0, "/opt/trn_rl_repo")

import numpy as np
import ml_dtypes

BF16 = ml_dtypes.bfloat16

H = 16       # total heads
D = 128      # head dim
R = 64       # rope part of head dim
HID = 2048   # hidden
DL = 512     # kv latent
S = 2048     # seq len
B = 2        # batch
EPS = 1e-6

TP = 4                # head-parallel ways
DP = 2                # batch-parallel ways
HPC = H // TP         # heads per core = 4
NKT = S // 128        # 16 k tiles of 128
NQB = S // 512        # 4 q blocks of 512
KT_HID = HID // 128   # 16 contraction tiles for HID
KT_DL = DL // 128     # 4 contraction tiles for DL

_PROGRAMS = {}


def _build_program(repeat=1, apply_g=False):
    import concourse.bass as bass
    from concourse import bacc, mybir
    import concourse.tile as tile

    f32 = mybir.dt.float32
    bf16 = mybir.dt.bfloat16
    Exp = mybir.ActivationFunctionType.Exp
    Sqrt = mybir.ActivationFunctionType.Sqrt

    nc = bacc.Bacc(
        "TRN2", target_bir_lowering=False, debug=False, enable_asserts=False
    )

    xT_d = nc.dram_tensor("xt", [HID, S], bf16, kind="ExternalInput").ap()
    wq_d = nc.dram_tensor("wq", [HID, HPC * D], bf16, kind="ExternalInput").ap()
    wkv_d = nc.dram_tensor("wkv", [HID, DL], bf16, kind="ExternalInput").ap()
    wk_d = nc.dram_tensor("wk", [DL, HPC * D], bf16, kind="ExternalInput").ap()
    wv_d = nc.dram_tensor("wv", [DL, HPC * D], bf16, kind="ExternalInput").ap()
    wo_d = nc.dram_tensor("wo", [HPC * D, HID], bf16, kind="ExternalInput").ap()
    onec_d = nc.dram_tensor("onecol", [128, 1], bf16, kind="ExternalInput").ap()
    cmask_d = nc.dram_tensor("cmask", [128, 4, 512], bf16, kind="ExternalInput").ap()
    out_d = nc.dram_tensor("out", [S, HID], f32, kind="ExternalOutput").ap()
    if apply_g:
        gqc_d = nc.dram_tensor("gqcol", [128, 1], f32, kind="ExternalInput").ap()
        gkc_d = nc.dram_tensor("gkcol", [128, 1], f32, kind="ExternalInput").ap()

    with tile.TileContext(nc) as tc:
        with (
            tc.tile_pool(name="consts", bufs=1) as consts,
            tc.tile_pool(name="acts", bufs=1) as acts,
        ):
            # eps biases for the rsqrt rows:
            #  q: rsqrt(sumsq + D*eps) == rsqrt(mean+eps)/sqrt(D)  (attn scale folded)
            #  k: rsqrt(sumsq/D + eps) == rsqrt(mean+eps)
            epsq_sb = consts.tile([1, 1], f32)
            nc.vector.memset(epsq_sb, EPS * D)
            epsk_sb = consts.tile([1, 1], f32)
            nc.vector.memset(epsk_sb, EPS)
            onec_sb = consts.tile_from(onec_d)
            cmask_sb = consts.tile_from(cmask_d)
            if apply_g:
                gqc_sb = consts.tile_from(gqc_d)
                gkc_sb = consts.tile_from(gkc_d)

            qT = acts.tile([128, HPC, S], bf16)
            kT = acts.tile([128, HPC, S], bf16)
            latT = acts.tile([128, KT_DL, S], bf16)
            v = acts.tile([128, NKT, HPC * D], bf16)
            oT = acts.tile([128, HPC, S], bf16)

            for _rep in range(repeat):
                # ------------ stage 1: x-projections ------------
                with (
                    tc.tile_pool(name="xw", bufs=1) as xw,
                    tc.tile_pool(name="ps1", bufs=1, space="PSUM") as ps1,
                ):
                    xT_sb = xw.tile([128, KT_HID, S], bf16)
                    wq_sb = xw.tile([128, KT_HID, HPC * D], bf16)
                    wkv_sb = xw.tile([128, KT_HID, DL], bf16)
                    xr = xT_d.rearrange("(t p) s -> p t s", p=128)
                    wqr = wq_d.rearrange("(t p) n -> p t n", p=128)
                    wkvr = wkv_d.rearrange("(t p) n -> p t n", p=128)
                    for k in range(KT_HID):
                        nc.sync.dma_start(out=wq_sb[:, k, :], in_=wqr[:, k, :])
                        nc.sync.dma_start(out=wkv_sb[:, k, :], in_=wkvr[:, k, :])
                        nc.sync.dma_start(out=xT_sb[:, k, :], in_=xr[:, k, :])

                    for sb in range(NQB):
                        pq = [
                            ps1.tile([128, 512], f32, name=f"pq{n}", tag=f"pq{n}")
                            for n in range(HPC)
                        ]
                        pl = [
                            ps1.tile([128, 512], f32, name=f"pl{n}", tag=f"pl{n}")
                            for n in range(HPC)
                        ]
                        ss = slice(sb * 512, (sb + 1) * 512)
                        for k in range(KT_HID):
                            st, sp = (k == 0), (k == KT_HID - 1)
                            for n in range(HPC):
                                nc.tensor.matmul(
                                    pq[n],
                                    lhsT=wq_sb[:, k, n * 128 : (n + 1) * 128],
                                    rhs=xT_sb[:, k, ss],
                                    start=st, stop=sp,
                                )
                            for n in range(HPC):
                                nc.tensor.matmul(
                                    pl[n],
                                    lhsT=wkv_sb[:, k, n * 128 : (n + 1) * 128],
                                    rhs=xT_sb[:, k, ss],
                                    start=st, stop=sp,
                                )
                        for n in range(HPC):
                            nc.scalar.copy(out=qT[:, n, ss], in_=pq[n])
                            nc.vector.tensor_copy(out=latT[:, n, ss], in_=pl[n])

                # weights for later stages: loaded into space freed by stage 1
                with (
                    tc.tile_pool(name="w2", bufs=1) as w2,
                    tc.tile_pool(name="work", bufs=4) as work,
                    tc.tile_pool(name="nrow", bufs=4) as nrow,
                    tc.tile_pool(name="bcp", bufs=4) as bcp,
                ):
                    wk_sb = w2.tile_from(wk_d.rearrange("(t p) n -> p t n", p=128))
                    wv_sb = w2.tile_from(wv_d.rearrange("(t p) n -> p t n", p=128))
                    wo_sb = w2.tile_from(wo_d.rearrange("(t p) n -> p t n", p=128))

                    # ------ stage 2+3: latent projections + q/k RMSNorm ------
                    with (
                        tc.tile_pool(name="psk2", bufs=1, space="PSUM") as psk2,
                        tc.tile_pool(name="psv2", bufs=1, space="PSUM") as psv2,
                        tc.tile_pool(name="psn", bufs=2, space="PSUM") as psn,
                    ):
                        def norm_block(t_sb, sb, which):
                            ss = slice(sb * 512, (sb + 1) * 512)
                            eps_b = epsq_sb if which == "gq" else epsk_sb
                            scale = 1.0 if which == "gq" else 1.0 / D
                            for h in range(HPC):
                                sl = t_sb[:, h, ss]
                                sq = work.tile([128, 512], bf16, tag="sq")
                                nc.vector.tensor_mul(sq, sl, sl)
                                pss = psn.tile([1, 512], f32, name="pss")
                                nc.tensor.matmul(
                                    pss, lhsT=onec_sb, rhs=sq,
                                    start=True, stop=True,
                                )
                                row = nrow.tile([1, 512], f32, tag="row")
                                nc.scalar.activation(
                                    out=row, in_=pss, func=Sqrt,
                                    bias=eps_b, scale=scale,
                                )
                                nc.vector.reciprocal(out=row, in_=row)
                                bc = bcp.tile([128, 512], f32, tag="bc")
                                nc.gpsimd.partition_broadcast(bc, row)
                                nc.vector.tensor_mul(sl, sl, bc)
                                if apply_g:
                                    gc = gqc_sb if which == "gq" else gkc_sb
                                    nc.vector.tensor_scalar_mul(sl, sl, gc)

                        # per sb: q-norm (ready since stage 1), kT+v matmuls,
                        # then k-norm — PE filler comes from the next sb's MMs
                        for sb in range(NQB):
                            norm_block(qT, sb, "gq")
                            pk = [
                                psk2.tile([128, 512], f32, name=f"pk{h}", tag=f"pk{h}")
                                for h in range(HPC)
                            ]
                            ss = slice(sb * 512, (sb + 1) * 512)
                            for c in range(KT_DL):
                                for h in range(HPC):
                                    nc.tensor.matmul(
                                        pk[h],
                                        lhsT=wk_sb[:, c, h * 128 : (h + 1) * 128],
                                        rhs=latT[:, c, ss],
                                        start=(c == 0), stop=(c == KT_DL - 1),
                                    )
                            for h in range(HPC):
                                nc.scalar.copy(out=kT[:, h, ss], in_=pk[h])
                            # v for this sb's 4 s-tiles
                            for st in range(4 * sb, 4 * sb + 4):
                                pv = psv2.tile(
                                    [128, 512], f32, name=f"pv{st % 2}",
                                    tag=f"pv{st % 2}",
                                )
                                for c in range(KT_DL):
                                    nc.tensor.matmul(
                                        pv,
                                        lhsT=latT[:, c, st * 128 : (st + 1) * 128],
                                        rhs=wv_sb[:, c, :],
                                        start=(c == 0), stop=(c == KT_DL - 1),
                                    )
                                nc.vector.tensor_copy(out=v[:, st, :], in_=pv)
                            norm_block(kT, sb, "gk")

                    # ------------ stages 4+5: attention + out projection ----
                    with (
                        tc.tile_pool(name="epool", bufs=8) as epool,
                        tc.tile_pool(name="otile", bufs=4) as otile,
                        tc.tile_pool(name="posb", bufs=3) as posb,
                        tc.tile_pool(name="pss", bufs=2, space="PSUM") as pss_p,
                        tc.tile_pool(name="pso", bufs=2, space="PSUM") as pso_p,
                        tc.tile_pool(name="pssum", bufs=2, space="PSUM") as pssum_p,
                        tc.tile_pool(name="ps5", bufs=1, space="PSUM") as ps5,
                    ):
                        def out_block(st):
                            # output projection for one s-tile of oT
                            for nbp in range(2):  # nb pairs share 2 psum banks
                                px = [
                                    ps5.tile(
                                        [128, 512], f32,
                                        name=f"px{i}", tag=f"px{i}",
                                    )
                                    for i in range(2)
                                ]
                                for c in range(HPC):
                                    for i in range(2):
                                        nb = 2 * nbp + i
                                        nc.tensor.matmul(
                                            px[i],
                                            lhsT=oT[:, c, st * 128 : (st + 1) * 128],
                                            rhs=wo_sb[:, c, nb * 512 : (nb + 1) * 512],
                                            start=(c == 0), stop=(c == HPC - 1),
                                        )
                                for i in range(2):
                                    nb = 2 * nbp + i
                                    ot = otile.tile(
                                        [128, 512], f32, name=f"ot{i}",
                                        tag=f"ot{i}",
                                    )
                                    if i == 0:
                                        nc.vector.tensor_copy(out=ot, in_=px[i])
                                    else:
                                        nc.scalar.copy(out=ot, in_=px[i])
                                    nc.sync.dma_start(
                                        out=out_d[
                                            st * 128 : (st + 1) * 128,
                                            nb * 512 : (nb + 1) * 512,
                                        ],
                                        in_=ot,
                                    )

                        for j in range(NQB):  # q block of 512
                            nk = 4 * j + 4  # valid k tiles (causal)
                            js = slice(j * 512, (j + 1) * 512)
                            for h in range(HPC):
                                po = pso_p.tile([128, 512], f32, name="po")
                                psum = pssum_p.tile([1, 512], f32, name="psum")
                                qs = qT[:, h, js]
                                for t in range(nk):
                                    ps = pss_p.tile([128, 512], f32, name="ps")
                                    nc.tensor.matmul(
                                        ps,
                                        lhsT=kT[:, h, t * 128 : (t + 1) * 128],
                                        rhs=qs, start=True, stop=True,
                                    )
                                    e = epool.tile([128, 512], bf16, tag="e")
                                    nc.scalar.activation(out=e, in_=ps, func=Exp)
                                    m = t - 4 * j
                                    if m >= 0:  # diagonal band: causal mask
                                        nc.vector.tensor_mul(e, e, cmask_sb[:, m, :])
                                    nc.tensor.matmul(
                                        psum, lhsT=onec_sb, rhs=e,
                                        start=(t == 0), stop=(t == nk - 1),
                                    )
                                    nc.tensor.matmul(
                                        po,
                                        lhsT=v[:, t, h * 128 : (h + 1) * 128],
                                        rhs=e, start=(t == 0), stop=(t == nk - 1),
                                    )
                                # copy po out of PSUM promptly (frees the bank
                                # without waiting on the recip/broadcast chain)
                                posbt = posb.tile([128, 512], f32, tag="posb")
                                nc.scalar.copy(out=posbt, in_=po)
                                srow = nrow.tile([1, 512], f32, tag="sr")
                                nc.vector.reciprocal(out=srow, in_=psum)
                                bc = bcp.tile([128, 512], f32, tag="abc")
                                nc.gpsimd.partition_broadcast(bc, srow)
                                nc.vector.tensor_mul(oT[:, h, js], posbt, bc)
                                # interleave previous q-block's out projection so
                                # PE has filler work across block boundaries
                                if j > 0:
                                    out_block(4 * (j - 1) + h)
                        for st in range(4 * (NQB - 1), 4 * NQB):
                            out_block(st)

    nc.compile()
    return nc


def _get_program(repeat=1, apply_g=False):
    key = (repeat, apply_g)
    if key not in _PROGRAMS:
        _PROGRAMS[key] = _build_program(repeat, apply_g)
    return _PROGRAMS[key]


def _host_prep(x, Wq, Wkv, Wkr, Wkn, Wv, Wo, gq, gk):
    """Build the 8 per-core input maps. Returns (in_maps, apply_g)."""
    x = np.asarray(x, np.float32)
    gq = np.asarray(gq, np.float32)
    gk = np.asarray(gk, np.float32)
    apply_g = not (np.allclose(gq, 1.0) and np.allclose(gk, 1.0))

    # Fuse Wkr/Wkn so head h's kT block is 128 contiguous cols.
    Wk = np.empty((DL, H * D), np.float32)
    Wkr = np.asarray(Wkr, np.float32).reshape(DL, H, R)
    Wkn = np.asarray(Wkn, np.float32).reshape(DL, H, D - R)
    Wk_r = Wk.reshape(DL, H, D)
    Wk_r[:, :, :R] = Wkr
    Wk_r[:, :, R:] = Wkn

    Wq16 = np.ascontiguousarray(np.asarray(Wq, np.float32)).astype(BF16)
    Wkv16 = np.ascontiguousarray(np.asarray(Wkv, np.float32)).astype(BF16)
    Wk16 = Wk.astype(BF16)
    Wv16 = np.ascontiguousarray(np.asarray(Wv, np.float32)).astype(BF16)
    Wo16 = np.ascontiguousarray(np.asarray(Wo, np.float32)).astype(BF16)

    xT16 = [np.ascontiguousarray(x[b].T).astype(BF16) for b in range(B)]

    onec = np.ones((128, 1), BF16)
    kk = np.arange(128).reshape(128, 1, 1)
    mm = np.arange(4).reshape(1, 4, 1)
    qq = np.arange(512).reshape(1, 1, 512)
    cmask = (qq >= kk + 128 * mm).astype(BF16)  # [128, 4, 512]

    in_maps = []
    for core in range(8):
        tp, dp = core % TP, core // TP
        hs = slice(tp * HPC * D, (tp + 1) * HPC * D)
        m = {
            "xt": xT16[dp],
            "wq": np.ascontiguousarray(Wq16[:, hs]),
            "wkv": Wkv16,
            "wk": np.ascontiguousarray(Wk16[:, hs]),
            "wv": np.ascontiguousarray(Wv16[:, hs]),
            "wo": np.ascontiguousarray(Wo16[hs, :]),
            "onecol": onec,
            "cmask": cmask,
        }
        if apply_g:
            m["gqcol"] = gq.reshape(128, 1).astype(np.float32)
            m["gkcol"] = gk.reshape(128, 1).astype(np.float32)
        in_maps.append(m)
    return in_maps, apply_g


def kernel(x, Wq, Wkv, Wkr, Wkn, Wv, Wo, gq, gk, _trace=False):
    from concourse import bass_utils

    in_maps, apply_g = _host_prep(x, Wq, Wkv, Wkr, Wkn, Wv, Wo, gq, gk)
    nc = _get_program(1, apply_g)
    res = bass_utils.run_bass_kernel_spmd(
        nc, in_maps, core_ids=list(range(8)), trace=_trace
    )
    out = np.zeros((B, S, HID), np.float32)
    for core in range(8):
        out[core // TP] += np.asarray(res.results[core]["out"], np.float32)
    if _trace:
        kernel._last_result = res
    return out
